# revision 1
# baseline (speedup 1.0000x reference)
"""Trainium2 Bass kernel for nn_EpisodicAdapter (GQA attention with LoRA adapters).

Sharding: Megatron-style tensor parallel over 8 NeuronCores.
  - core c owns query heads [4c..4c+4) (512 q-dims) and kv head c (128 dims)
  - Q/K/V projections column-sharded; attention head-sharded (no resharding)
  - context AllGather'd (per batch) in bf16, o_proj column-sharded so each
    core produces a 512-column slice of the output; host concatenates.

LoRA is folded on the host: x@W + s*(x@A)@B == x@(W + s*A@B), so the device
only sees effective weights (exact up to fp32 rounding).

All big matmuls run in bf16 (1 cyc/row on the PE vs 4 for fp32); accumulation
is fp32 in PSUM; softmax runs in fp32 on the scalar engine.

Schedule: per batch b -> [proj chunks 2b,2b+1 -> v transposes -> attention ->
AllGather_b], then the column-sharded o_proj for all batches. The per-batch
AllGathers overlap with the next batch's projection/attention compute.

Attention math per (batch, head) in transposed layout (d on partitions):
  scoresT[t,q] = kT[:,t].T @ qT       (PE, one 128-deep pass)
  expT = exp(scoresT/sqrt(128))       (ACT, psum->sbuf bf16)
  ctxT[d,q]   = sum_t v[t,d] expT     (PE accumulate, v as stationary)
  Z[1,q]      = sum_t 1    expT       (PE with ones lhsT)
  ctxT_norm   = ctxT * bcast(1/Z)     (DVE recip + PE K=1 fp32r broadcast)
The per-query softmax normalizer lands on the free axis in this layout, so it
is broadcast across partitions with a K=1 matmul instead of a transpose. The
scores/ctx matmuls are software-pipelined (ctx for tile tt-1 issues after
scores for tt) so the PE never waits on the ACT exp stream.

build_nc(reps=N) statically repeats the whole computation N times in one NEFF
(used by the timing harness to cancel dispatch overhead; the graded path uses
reps=1).
"""

import numpy as np
import ml_dtypes

import concourse.bass as bass
import concourse.mybir as mybir
import concourse.tile as tile
from concourse import bacc
from concourse.bass_utils import run_bass_kernel_spmd
from concourse.masks import make_identity

B, T, H = 4, 1024, 4096
NH, NKV, HD, R = 32, 8, 128, 16
SCALING = 32.0 / 16.0
NCORE = 8
TOK = B * T            # 4096 tokens
DQ = H // NCORE        # 512 query dims per core
HQ = DQ // HD          # 4 query heads per core
CH = 512               # token chunk for projections
NCH = TOK // CH
HT = H // 128          # 32 contraction tiles
ISCALE = float(1.0 / np.sqrt(HD))
NT = T // 128          # 8 key/value tiles per batch
NQC = T // CH          # 2 query chunks per batch

BF16 = mybir.dt.bfloat16
F32 = mybir.dt.float32
F32R = mybir.dt.float32r
NPBF = ml_dtypes.bfloat16


def build_nc(use_collective=True, reps=1, pipelined_oproj=True):
    nc = bacc.Bacc("TRN2", target_bir_lowering=False, debug=False,
                   num_devices=NCORE if use_collective else 1)

    hsT = nc.dram_tensor("hsT", [H, TOK], BF16, kind="ExternalInput")
    trT = nc.dram_tensor("trT", [H, TOK], BF16, kind="ExternalInput")
    wq = nc.dram_tensor("wq", [128, HT * DQ], BF16, kind="ExternalInput")
    wk = nc.dram_tensor("wk", [128, HT * HD], BF16, kind="ExternalInput")
    wv = nc.dram_tensor("wv", [128, HT * HD], BF16, kind="ExternalInput")
    wo = nc.dram_tensor("wo", [128, HT * DQ], BF16, kind="ExternalInput")
    hres = nc.dram_tensor("hres", [TOK, DQ], F32, kind="ExternalInput")
    out = nc.dram_tensor("out", [TOK, DQ], F32, kind="ExternalOutput")

    with tile.TileContext(nc) as tc:
        with (
            tc.tile_pool(name="dram", bufs=1, space="DRAM") as dram_pool,
            tc.tile_pool(name="const", bufs=1) as const_pool,
            tc.tile_pool(name="qkv", bufs=1) as qkv_pool,
            tc.tile_pool(name="w1", bufs=1) as w1,
            tc.tile_pool(name="x1", bufs=5) as x1,
            tc.tile_pool(name="a2", bufs=2) as a2,
            tc.tile_pool(name="psA", bufs=1, space="PSUM") as psA,
            tc.tile_pool(name="x3", bufs=4) as x3,
            tc.tile_pool(name="s3", bufs=2) as s3,
        ):
            ones_col = const_pool.tile([128, 1], BF16, name="ones_col")
            nc.vector.memset(ones_col[:], 1.0)
            ones_f32 = const_pool.tile([1, 128], F32, name="ones_f32")
            nc.vector.memset(ones_f32[:], 1.0)
            ones_row = const_pool.tile([1, 128], F32R, name="ones_row")
            nc.vector.tensor_copy(ones_row[:], ones_f32[:])
            ident = const_pool.tile([128, 128], BF16, name="ident")
            make_identity(nc, ident[:])

            qT = qkv_pool.tile([128, HQ * TOK], BF16, name="qT")
            kT = qkv_pool.tile([128, TOK], BF16, name="kT")
            vT = qkv_pool.tile([128, TOK], BF16, name="vT")
            vN = qkv_pool.tile([128, TOK], BF16, name="vN")

            # only the first wq slice + wk/wv are startup-critical; the later
            # wq slices are emitted after the first activation tiles so they
            # don't crowd the DMA queues during PE ramp-up
            wq_sb = w1.tile([128, HT * DQ], BF16, name="wq_sb")
            nc.sync.dma_start(out=wq_sb[:, 0:(HT // 4) * DQ],
                              in_=wq[:, 0:(HT // 4) * DQ])
            wk_sb = w1.tile([128, HT * HD], BF16, name="wk_sb")
            nc.sync.dma_start(out=wk_sb[:], in_=wk[:])
            wv_sb = w1.tile([128, HT * HD], BF16, name="wv_sb")
            nc.sync.dma_start(out=wv_sb[:], in_=wv[:])
            wo_sb = w1.tile([128, HT * DQ], BF16, name="wo_sb")

            def emit_oproj(p, b, ag_out):
                """Column-sharded o_proj + residual for one batch."""
                for blk in range(NQC):
                    o_ps = [psA.tile([128, CH], F32, name=f"{p}o_{b}_{blk}_{tt}",
                                     tag=f"bA{tt + 4 * (blk % 2)}")
                            for tt in range(4)]
                    for fr in range(HT):
                        ct_t = x3.tile([128, CH], BF16, name=f"{p}ct_{b}_{blk}_{fr}",
                                       tag="ct", bufs=16)
                        nc.sync.dma_start(
                            out=ct_t[:],
                            in_=ag_out[b][fr * 128:(fr + 1) * 128,
                                          blk * CH:(blk + 1) * CH])
                        for tt in range(4):
                            nc.tensor.matmul(
                                o_ps[tt][:], ct_t[:, tt * 128:(tt + 1) * 128],
                                wo_sb[:, fr * DQ:(fr + 1) * DQ],
                                start=(fr == 0), stop=(fr == HT - 1))
                    for tt in range(4):
                        rows = b * T + blk * CH + tt * 128
                        res_t = x3.tile([128, CH], F32, name=f"{p}res_{b}_{blk}_{tt}",
                                        tag="res", bufs=6)
                        nc.sync.dma_start(out=res_t[:],
                                          in_=hres[rows: rows + 128, :])
                        o_sb = s3.tile([128, CH], F32, name=f"{p}ob_{b}_{blk}_{tt}",
                                       tag="o_sb", bufs=3)
                        nc.vector.tensor_add(o_sb[:], o_ps[tt][:], res_t[:])
                        nc.sync.dma_start(out=out[rows: rows + 128, :], in_=o_sb[:])

            for rp in range(reps):
                p = f"r{rp}_"
                ag_in = [dram_pool.tile([DQ, T], BF16, name=f"{p}ag_in{b}")
                         for b in range(B)]
                ag_out = [
                    dram_pool.tile([NCORE * DQ, T], BF16,
                                   addr_space="Shared" if use_collective else "Local",
                                   name=f"{p}ag_out{b}")
                    for b in range(B)
                ]
                for b in range(B):
                    # ---- projections for this batch's two token chunks ----
                    for ch in (2 * b, 2 * b + 1):
                        q_ps = [psA.tile([128, CH], F32, name=f"{p}q_ps{j}_{ch}",
                                         tag=f"bA{j}") for j in range(HQ)]
                        k_ps = psA.tile([128, CH], F32, name=f"{p}k_ps_{ch}", tag="bA4")
                        v_ps = psA.tile([128, CH], F32, name=f"{p}v_ps_{ch}", tag="bA5")
                        for ht in range(HT):
                            hs_t = x1.tile([128, CH], BF16, name=f"{p}hs_{ch}_{ht}",
                                           tag="hs_t")
                            nc.sync.dma_start(
                                out=hs_t[:],
                                in_=hsT[ht * 128:(ht + 1) * 128, ch * CH:(ch + 1) * CH])
                            tr_t = x1.tile([128, CH], BF16, name=f"{p}tr_{ch}_{ht}",
                                           tag="tr_t")
                            nc.sync.dma_start(
                                out=tr_t[:],
                                in_=trT[ht * 128:(ht + 1) * 128, ch * CH:(ch + 1) * CH])
                            if rp == 0 and ch == 0 and ht in (2, 4, 6):
                                s = ht // 2  # deferred wq slices 1..3
                                lo, hi = s * (HT // 4) * DQ, (s + 1) * (HT // 4) * DQ
                                nc.sync.dma_start(out=wq_sb[:, lo:hi],
                                                  in_=wq[:, lo:hi])
                            st = ht == 0
                            en = ht == HT - 1
                            for j in range(HQ):
                                nc.tensor.matmul(
                                    q_ps[j][:],
                                    wq_sb[:, ht * DQ + j * 128: ht * DQ + (j + 1) * 128],
                                    hs_t[:], start=st, stop=en)
                            nc.tensor.matmul(k_ps[:], wk_sb[:, ht * HD:(ht + 1) * HD],
                                             tr_t[:], start=st, stop=en)
                            nc.tensor.matmul(v_ps[:], wv_sb[:, ht * HD:(ht + 1) * HD],
                                             tr_t[:], start=st, stop=en)
                        for j in range(HQ):
                            nc.vector.tensor_copy(
                                qT[:, j * TOK + ch * CH: j * TOK + (ch + 1) * CH],
                                q_ps[j][:])
                        nc.vector.tensor_copy(kT[:, ch * CH:(ch + 1) * CH], k_ps[:])
                        nc.vector.tensor_copy(vT[:, ch * CH:(ch + 1) * CH], v_ps[:])

                    # ---- transpose this batch's v tiles to natural layout ----
                    for i in range(b * NT, (b + 1) * NT):
                        # z banks: released early by the previous attention,
                        # and handed back before this batch's z allocations
                        tp_ps = psA.tile([128, 128], BF16, name=f"{p}tp_{i}",
                                         tag=f"bA{2 + (i % 2)}")
                        nc.tensor.transpose(tp_ps[:], vT[:, i * 128:(i + 1) * 128],
                                            ident[:])
                        nc.vector.tensor_copy(vN[:, i * 128:(i + 1) * 128], tp_ps[:])

                    # ---- attention for this batch ----
                    for hq in range(HQ):
                        expT = a2.tile([128, NT * T], BF16,
                                       name=f"{p}expT_{b}_{hq}", tag="expT", bufs=1)
                        for qc in range(NQC):
                            # ctx on bA4/5 (released latest, needed latest by
                            # the next projection chunk); z on bA2/3 (released
                            # early, right after the z_sb copy)
                            ctx_ps = psA.tile([128, CH], F32,
                                              name=f"{p}ctx_{b}_{hq}_{qc}",
                                              tag=f"bA{4 + (qc % 2)}")
                            z_tag = f"bA{2 + (qc % 2)}"
                            z_ps = psA.tile([1, CH], F32, name=f"{p}z_{b}_{hq}_{qc}",
                                            tag=z_tag)

                            def sc_mm(tt):
                                # 4-bank rotation so the trailing exp reads
                                # never gate the next phase's allocations
                                sc_ps = psA.tile([128, CH], F32,
                                                 name=f"{p}sc_{b}_{hq}_{qc}_{tt}",
                                                 tag=f"bA{(0, 1, 6, 7)[tt % 4]}")
                                nc.tensor.matmul(
                                    sc_ps[:],
                                    kT[:, b * T + tt * 128: b * T + (tt + 1) * 128],
                                    qT[:, hq * TOK + b * T + qc * CH:
                                       hq * TOK + b * T + (qc + 1) * CH],
                                    start=True, stop=True)
                                ex = expT[:, tt * T + qc * CH: tt * T + (qc + 1) * CH]
                                nc.scalar.activation(
                                    ex, sc_ps[:], mybir.ActivationFunctionType.Exp,
                                    scale=ISCALE)
                                return ex

                            def cz_mm(tt, ex):
                                nc.tensor.matmul(
                                    ctx_ps[:],
                                    vN[:, (b * NT + tt) * 128:(b * NT + tt + 1) * 128],
                                    ex, start=(tt == 0), stop=(tt == NT - 1))
                                nc.tensor.matmul(
                                    z_ps[:], ones_col[:], ex,
                                    start=(tt == 0), stop=(tt == NT - 1))

                            # software pipeline: scores(tt) ahead of ctx/z(tt-1)
                            exs = [sc_mm(0)]
                            for tt in range(1, NT):
                                exs.append(sc_mm(tt))
                                cz_mm(tt - 1, exs[tt - 1])
                            cz_mm(NT - 1, exs[NT - 1])

                            # normalize: broadcast Z across partitions (K=1
                            # fp32r matmul), reciprocal to SBUF, then scale
                            z_sb = a2.tile([1, CH], F32R,
                                           name=f"{p}zs_{b}_{hq}_{qc}", tag="z_sb")
                            nc.vector.tensor_copy(z_sb[:], z_ps[:])
                            # zb reuses z's bank: the z->copy->zb chain is
                            # serial anyway, and this frees bA6/7 for scores
                            zb_ps = psA.tile([128, CH], F32,
                                             name=f"{p}zb_{b}_{hq}_{qc}",
                                             tag=z_tag)
                            nc.tensor.matmul(zb_ps[:], ones_row[:], z_sb[:],
                                             start=True, stop=True)
                            recip = a2.tile([128, CH], F32,
                                            name=f"{p}rc_{b}_{hq}_{qc}", tag="recip")
                            nc.vector.reciprocal(recip[:], zb_ps[:])
                            ctxn = a2.tile([128, CH], BF16,
                                           name=f"{p}ctxn_{b}_{hq}_{qc}", tag="ctxn")
                            nc.vector.tensor_mul(ctxn[:], ctx_ps[:], recip[:])
                            nc.sync.dma_start(
                                out=ag_in[b][hq * 128:(hq + 1) * 128,
                                             qc * CH:(qc + 1) * CH],
                                in_=ctxn[:])
                    if use_collective:
                        nc.gpsimd.collective_compute(
                            "AllGather",
                            mybir.AluOpType.bypass,
                            replica_groups=[list(range(NCORE))],
                            ins=[ag_in[b].opt()],
                            outs=[ag_out[b].opt()],
                        )
                    else:
                        # single-core timing stand-in: tiny copy just to create
                        # the dependency edge (the real AllGather runs on the
                        # TOPSP/SDMA silicon, not on the engine DMA queues)
                        nc.sync.dma_start(out=ag_out[b][0:16, 0:16],
                                          in_=ag_in[b][0:16, 0:16])
                    if rp == 0 and b == 0:
                        nc.sync.dma_start(out=wo_sb[:], in_=wo[:])
                    if pipelined_oproj and b >= 1:
                        # o_proj pipelined one batch behind: spreads the 33MB
                        # gathered-ctx DMA across the run and gives each
                        # AllGather a full batch of slack
                        emit_oproj(p, b - 1, ag_out)
                if pipelined_oproj:
                    emit_oproj(p, B - 1, ag_out)
                else:
                    for b in range(B):
                        emit_oproj(p, b, ag_out)

    nc.compile()
    return nc


def _tile_w(w):
    """[H, O] row-major -> [128, HT*O] so [:, ht*O:(ht+1)*O] is rows ht*128..+128."""
    Hh, O = w.shape
    return np.ascontiguousarray(
        w.reshape(Hh // 128, 128, O).transpose(1, 0, 2).reshape(128, (Hh // 128) * O)
    ).astype(NPBF)


def make_in_maps(inputs):
    hs = np.asarray(inputs["hidden_states"], np.float32).reshape(TOK, H)
    tr = np.asarray(inputs["traces"], np.float32).reshape(TOK, H)
    hsT = np.ascontiguousarray(hs.T).astype(NPBF)
    trT = np.ascontiguousarray(tr.T).astype(NPBF)

    def eff(Wname, Aname, Bname):
        W = np.asarray(inputs[Wname], np.float32)
        A = np.asarray(inputs[Aname], np.float32)
        Bm = np.asarray(inputs[Bname], np.float32)
        return W + np.float32(SCALING) * (A @ Bm)

    Wq = eff("Wq", "Aq", "Bq")
    Wk = eff("Wk", "Ak", "Bk")
    Wv = eff("Wv", "Av", "Bv")
    Wo = eff("Wo", "Ao", "Bo")

    in_maps = []
    for c in range(NCORE):
        in_maps.append({
            "hsT": hsT,
            "trT": trT,
            "wq": _tile_w(Wq[:, c * DQ:(c + 1) * DQ]),
            "wk": _tile_w(Wk[:, c * HD:(c + 1) * HD]),
            "wv": _tile_w(Wv[:, c * HD:(c + 1) * HD]),
            "wo": _tile_w(Wo[:, c * DQ:(c + 1) * DQ]),
            "hres": np.ascontiguousarray(hs[:, c * DQ:(c + 1) * DQ]),
        })
    return in_maps


_NC_CACHE = {}


def _get_runner():
    """Cached jitted 8-core runner (mirrors bass2jax.run_bass_via_pjrt but
    reuses the jit across kernel() calls)."""
    if "runner" in _NC_CACHE:
        return _NC_CACHE["runner"]
    import jax
    from jax.sharding import Mesh, PartitionSpec, NamedSharding
    from jax.experimental.shard_map import shard_map
    import concourse.mybir as mb
    from concourse import bass2jax

    nc = _NC_CACHE.get("nc")
    if nc is None:
        nc = _NC_CACHE["nc"] = build_nc(use_collective=True)
    bass2jax.install_neuronx_cc_hook()
    partition_name = nc.partition_id_tensor.name if nc.partition_id_tensor else None
    in_names, out_names, out_avals, zero_outs = [], [], [], []
    for alloc in nc.m.functions[0].allocations:
        if not isinstance(alloc, mb.MemoryLocationSet):
            continue
        name = alloc.memorylocations[0].name
        if alloc.kind == "ExternalInput":
            if name != partition_name:
                in_names.append(name)
        elif alloc.kind == "ExternalOutput":
            out_names.append(name)
            shape = tuple(alloc.tensor_shape)
            dtype = mb.dt.np(alloc.dtype)
            out_avals.append(jax.core.ShapedArray(shape, dtype))
            zero_outs.append(np.zeros(shape, dtype))
    all_names = in_names + out_names
    if partition_name is not None:
        all_names = all_names + [partition_name]

    def _body(*args):
        operands = list(args)
        if partition_name is not None:
            operands.append(bass2jax.partition_id_tensor())
        outs = bass2jax._bass_exec_p.bind(
            *operands,
            out_avals=tuple(out_avals),
            in_names=tuple(all_names),
            out_names=tuple(out_names),
            lowering_input_output_aliases=(),
            sim_require_finite=True,
            sim_require_nnan=True,
            nc=nc,
        )
        return tuple(outs)

    devices = jax.devices()[:NCORE]
    mesh = Mesh(np.asarray(devices), ("core",))
    spec = PartitionSpec("core")
    fn = jax.jit(shard_map(_body, mesh=mesh,
                           in_specs=(spec,) * (len(in_names) + len(out_names)),
                           out_specs=(spec,) * len(out_names), check_rep=False))
    sharding = NamedSharding(mesh, spec)
    zeros_dev = [
        jax.device_put(np.zeros((NCORE * z.shape[0], *z.shape[1:]), z.dtype), sharding)
        for z in zero_outs
    ]
    runner = {"fn": fn, "in_names": in_names, "out_names": out_names,
              "zeros": zeros_dev, "sharding": sharding, "jax": jax}
    _NC_CACHE["runner"] = runner
    return runner


def kernel(**inputs) -> np.ndarray:
    r = _get_runner()
    in_maps = make_in_maps(inputs)
    jax = r["jax"]
    args = [
        jax.device_put(
            np.concatenate([np.asarray(m[name]) for m in in_maps], axis=0),
            r["sharding"])
        for name in r["in_names"]
    ] + r["zeros"]
    outs = r["fn"](*args)
    oi = r["out_names"].index("out")
    full = np.asarray(outs[oi]).reshape(NCORE, TOK, DQ)
    out_full = np.empty((TOK, H), np.float32)
    for c in range(NCORE):
        out_full[:, c * DQ:(c + 1) * DQ] = full[c]
    return out_full.reshape(B, T, H)



# revision 22
# speedup vs baseline: 3.5257x; 3.5257x over previous
"""Trainium2 Bass kernel for nn_EpisodicAdapter (GQA attention with LoRA adapters).

Sharding: Megatron-style tensor parallel over 8 NeuronCores.
  - core c owns query heads [4c..4c+4) (512 q-dims) and kv head c (128 dims)
  - Q/K/V projections column-sharded; attention head-sharded (no resharding)
  - context AllGather'd (per batch) in bf16, o_proj column-sharded so each
    core produces a 512-column slice of the output; host concatenates.

LoRA is folded on the host: x@W + s*(x@A)B == x@(W + s*A@B), so the device
only sees effective weights (exact up to fp32 rounding).

All big matmuls run in bf16 (1 cyc/row on the PE vs 4 for fp32); accumulation
is fp32 in PSUM; softmax runs in fp32 on the scalar engine.

Schedule: per batch b -> [proj chunks 2b,2b+1 -> v transposes -> attention ->
AllGather_b], then the column-sharded o_proj for all batches. The per-batch
AllGathers overlap with the next batch's projection/attention compute.

Attention math per (batch, head) in transposed layout (d on partitions):
  scoresT[t,q] = kT[:,t].T @ qT       (PE, one 128-deep pass)
  expT = exp(scoresT/sqrt(128))       (ACT, psum->sbuf bf16)
  ctxT[d,q]   = sum_t v[t,d] expT     (PE accumulate, v as stationary)
  S[p,q]      = sum_tt expT_tt[p,q]   (DVE chain, f32r accumulator)
  Z[128,q]    = J.T @ S               (PE, one all-ones f32r matmul: the
                                       partition reduction AND the broadcast
                                       across partitions in a single pass)
  ctxT_norm   = ctxT * recip(Z)       (DVE)
The per-query softmax normalizer lands on the free axis in this layout; the
t-tile summation runs on the (otherwise idle-ish) DVE so the PE pays one
512-cycle matmul per (head, q-chunk) instead of nine. The normalizer tail
(J-matmul/recip/mul/DMA) is software-pipelined one (head,q-chunk) iteration
behind so the PE never waits on the DVE chain. The scores/ctx matmuls are
pipelined the same way (ctx for tile tt-1 issues after scores for tt).

build_nc(reps=N) statically repeats the whole computation N times in one NEFF
(used by the timing harness to cancel dispatch overhead; the graded path uses
reps=1).
"""

import numpy as np
import ml_dtypes

import concourse.bass as bass
import concourse.mybir as mybir
import concourse.tile as tile
from concourse import bacc
from concourse.bass_utils import run_bass_kernel_spmd
from concourse.masks import make_identity

B, T, H = 4, 1024, 4096
NH, NKV, HD, R = 32, 8, 128, 16
SCALING = 32.0 / 16.0
NCORE = 8
TOK = B * T            # 4096 tokens
DQ = H // NCORE        # 512 query dims per core
HQ = DQ // HD          # 4 query heads per core
CH = 512               # token chunk for projections
NCH = TOK // CH
HT = H // 128          # 32 contraction tiles
ISCALE = float(1.0 / np.sqrt(HD))
NT = T // 128          # 8 key/value tiles per batch
NQC = T // CH          # 2 query chunks per batch

BF16 = mybir.dt.bfloat16
FP16 = mybir.dt.float16
F32 = mybir.dt.float32
F32R = mybir.dt.float32r
NPBF = ml_dtypes.bfloat16
EXP_BIAS = -3.0  # exp(s - 3): keeps exp in fp16 range; cancels in softmax


def build_nc(use_collective=True, reps=1, pipelined_oproj=True):
    nc = bacc.Bacc("TRN2", target_bir_lowering=False, debug=False,
                   num_devices=NCORE if use_collective else 1)

    # activations are chunk-tiled on the host ([128, (ch*HT+ht)*CH]) so one
    # DMA fetches several ht-tiles contiguously: fewer HWDGE descriptor slots
    # (625ns each), which otherwise saturate during the projection phase
    hsT = nc.dram_tensor("hsT", [128, NCH * HT * CH], BF16, kind="ExternalInput")
    trT = nc.dram_tensor("trT", [128, NCH * HT * CH], BF16, kind="ExternalInput")
    wq = nc.dram_tensor("wq", [128, HT * DQ], BF16, kind="ExternalInput")
    wk = nc.dram_tensor("wk", [128, HT * HD], BF16, kind="ExternalInput")
    wv = nc.dram_tensor("wv", [128, HT * HD], BF16, kind="ExternalInput")
    wo = nc.dram_tensor("wo", [128, HT * DQ], BF16, kind="ExternalInput")
    hres = nc.dram_tensor("hres", [TOK, DQ], F32, kind="ExternalInput")
    out = nc.dram_tensor("out", [TOK, DQ], F32, kind="ExternalOutput")

    with tile.TileContext(nc) as tc:
        with (
            tc.tile_pool(name="dram", bufs=1, space="DRAM") as dram_pool,
            tc.tile_pool(name="const", bufs=1) as const_pool,
            tc.tile_pool(name="qkv", bufs=1) as qkv_pool,
            tc.tile_pool(name="w1", bufs=1) as w1,
            tc.tile_pool(name="x1", bufs=5) as x1,
            tc.tile_pool(name="a2", bufs=2) as a2,
            tc.tile_pool(name="psA", bufs=1, space="PSUM") as psA,
            tc.tile_pool(name="x3", bufs=4) as x3,
            tc.tile_pool(name="s3", bufs=2) as s3,
        ):
            ident = const_pool.tile([128, 128], BF16, name="ident")
            make_identity(nc, ident[:])
            # memset can't write f32r directly: fill f32, copy-convert
            ones_f32 = const_pool.tile([128, 128], F32, name="ones_f32")
            nc.vector.memset(ones_f32[:], 1.0)
            ones_J = const_pool.tile([128, 128], F32R, name="ones_J")
            nc.vector.tensor_copy(ones_J[:], ones_f32[:])
            exp_bias = const_pool.tile([128, 1], F32, name="exp_bias")
            nc.vector.memset(exp_bias[:], EXP_BIAS)

            qT = qkv_pool.tile([128, HQ * TOK], BF16, name="qT")
            kT = qkv_pool.tile([128, TOK], BF16, name="kT")
            vT = qkv_pool.tile([128, TOK], BF16, name="vT")
            # vN and the exp tiles are fp16: same PE/DVE rates as bf16 but a
            # 10-bit mantissa, so the softmax sums and ctx matmul are more
            # accurate; exp carries a -3 bias so e^s fits fp16 range
            vN = qkv_pool.tile([128, TOK], FP16, name="vN")

            wq_sb = w1.tile([128, HT * DQ], BF16, name="wq_sb")
            wk_sb = w1.tile([128, HT * HD], BF16, name="wk_sb")
            wv_sb = w1.tile([128, HT * HD], BF16, name="wv_sb")
            wo_sb = w1.tile([128, HT * DQ], BF16, name="wo_sb")

            def load_wq(lo, hi):
                nc.sync.dma_start(out=wq_sb[:, lo * DQ:hi * DQ],
                                  in_=wq[:, lo * DQ:hi * DQ])

            def load_wkv(lo, hi):
                nc.sync.dma_start(out=wk_sb[:, lo * HD:hi * HD],
                                  in_=wk[:, lo * HD:hi * HD])
                nc.sync.dma_start(out=wv_sb[:, lo * HD:hi * HD],
                                  in_=wv[:, lo * HD:hi * HD])

            # first-chunk weight streaming: interleave fine-grained weight
            # slices with the ht loop so the PE starts ~2us in instead of
            # waiting for 3MB of weights (the later chunks reuse SBUF copies)
            wq_sched = {0: (1, 4), 1: (4, 8), 2: (8, 16), 3: (16, 24),
                        5: (24, 32)}
            wkv_sched = {0: (4, 16), 2: (16, 32)}

            def emit_oproj(p, b, ag_out):
                """Column-sharded o_proj + residual for one batch."""
                for blk in range(NQC):
                    o_ps = [psA.tile([128, CH], F32, name=f"{p}o_{b}_{blk}_{tt}",
                                     tag=f"bA{tt + 4 * (blk % 2)}")
                            for tt in range(4)]
                    for fr in range(HT):
                        ct_t = x3.tile([128, CH], BF16, name=f"{p}ct_{b}_{blk}_{fr}",
                                       tag="ct", bufs=12)
                        nc.sync.dma_start(
                            out=ct_t[:],
                            in_=ag_out[b][blk][fr * 128:(fr + 1) * 128, :])
                        for tt in range(4):
                            nc.tensor.matmul(
                                o_ps[tt][:], ct_t[:, tt * 128:(tt + 1) * 128],
                                wo_sb[:, fr * DQ:(fr + 1) * DQ],
                                start=(fr == 0), stop=(fr == HT - 1))
                    for tt in range(4):
                        rows = b * T + blk * CH + tt * 128
                        res_t = x3.tile([128, CH], F32, name=f"{p}res_{b}_{blk}_{tt}",
                                        tag="res", bufs=4)
                        nc.sync.dma_start(out=res_t[:],
                                          in_=hres[rows: rows + 128, :])
                        o_sb = s3.tile([128, CH], F32, name=f"{p}ob_{b}_{blk}_{tt}",
                                       tag="o_sb", bufs=3)
                        nc.vector.tensor_add(o_sb[:], o_ps[tt][:], res_t[:])
                        nc.sync.dma_start(out=out[rows: rows + 128, :], in_=o_sb[:])

            for rp in range(reps):
                p = f"r{rp}_"
                # per (batch, q-chunk) gathers: the second half-batch gather
                # overlaps the first half's o_proj, so the tail o_proj starts
                # the moment attention ends
                ag_in = [[dram_pool.tile([DQ, CH], BF16, name=f"{p}ag_in{b}_{qc}")
                          for qc in range(NQC)] for b in range(B)]
                ag_out = [
                    [dram_pool.tile([NCORE * DQ, CH], BF16,
                                    addr_space="Shared" if use_collective else "Local",
                                    name=f"{p}ag_out{b}_{qc}")
                     for qc in range(NQC)]
                    for b in range(B)
                ]
                for b in range(B):
                    # ---- projections for this batch's two token chunks ----
                    for ch in (2 * b, 2 * b + 1):
                        first = rp == 0 and ch == 0
                        q_ps = [psA.tile([128, CH], F32, name=f"{p}q_ps{j}_{ch}",
                                         tag=f"bA{j}") for j in range(HQ)]
                        k_ps = psA.tile([128, CH], F32, name=f"{p}k_ps_{ch}", tag="bA4")
                        v_ps = psA.tile([128, CH], F32, name=f"{p}v_ps_{ch}", tag="bA5")
                        for g in range(HT // 2):
                            base = (ch * HT + 2 * g) * CH
                            hs_t = x1.tile([128, 2 * CH], BF16,
                                           name=f"{p}hs_{ch}_{g}", tag="hs_t",
                                           bufs=3)
                            nc.sync.dma_start(out=hs_t[:],
                                              in_=hsT[:, base:base + 2 * CH])
                            tr_t = x1.tile([128, 2 * CH], BF16,
                                           name=f"{p}tr_{ch}_{g}", tag="tr_t",
                                           bufs=3)
                            nc.sync.dma_start(out=tr_t[:],
                                              in_=trT[:, base:base + 2 * CH])
                            if first:
                                if g == 0:
                                    load_wq(0, 1)
                                    load_wkv(0, 4)
                                if g in wq_sched:
                                    load_wq(*wq_sched[g])
                                if g in wkv_sched:
                                    load_wkv(*wkv_sched[g])
                            for sub in range(2):
                                ht = 2 * g + sub
                                st = ht == 0
                                en = ht == HT - 1
                                hsx = hs_t[:, sub * CH:(sub + 1) * CH]
                                trx = tr_t[:, sub * CH:(sub + 1) * CH]
                                for j in range(HQ):
                                    nc.tensor.matmul(
                                        q_ps[j][:],
                                        wq_sb[:, ht * DQ + j * 128:
                                              ht * DQ + (j + 1) * 128],
                                        hsx, start=st, stop=en)
                                nc.tensor.matmul(
                                    k_ps[:], wk_sb[:, ht * HD:(ht + 1) * HD],
                                    trx, start=st, stop=en)
                                nc.tensor.matmul(
                                    v_ps[:], wv_sb[:, ht * HD:(ht + 1) * HD],
                                    trx, start=st, stop=en)
                        for j in range(HQ):
                            nc.vector.tensor_copy(
                                qT[:, j * TOK + ch * CH: j * TOK + (ch + 1) * CH],
                                q_ps[j][:])
                        nc.vector.tensor_copy(kT[:, ch * CH:(ch + 1) * CH], k_ps[:])
                        nc.vector.tensor_copy(vT[:, ch * CH:(ch + 1) * CH], v_ps[:])

                    # ---- transpose this batch's v tiles to natural layout ----
                    for i in range(b * NT, (b + 1) * NT):
                        # z banks: released early by the previous attention,
                        # and handed back before this batch's z allocations
                        tp_ps = psA.tile([128, 128], BF16, name=f"{p}tp_{i}",
                                         tag=f"bA{2 + (i % 2)}")
                        nc.tensor.transpose(tp_ps[:], vT[:, i * 128:(i + 1) * 128],
                                            ident[:])
                        nc.vector.tensor_copy(vN[:, i * 128:(i + 1) * 128], tp_ps[:])

                    # ---- attention for this batch ----
                    # qc-outer so each half-batch's context is complete (all
                    # heads) after 4 iterations and its AllGather can launch
                    # mid-attention. The normalizer tail of iteration i
                    # (J-matmul, recip, mul, ag_in DMA) is emitted during
                    # iteration i+1 so the PE never waits on the DVE S-chain.
                    pending = None
                    for qc in range(NQC):
                        for hq in range(HQ):
                            it = qc * HQ + hq
                            ex_t = a2.tile([128, NT * CH], FP16,
                                           name=f"{p}ex_{b}_{hq}_{qc}",
                                           tag="expT", bufs=2)
                            # ctx on bA4/5 (released latest, needed latest by
                            # the next projection chunk); z on bA2/3
                            ctx_ps = psA.tile([128, CH], F32,
                                              name=f"{p}ctx_{b}_{hq}_{qc}",
                                              tag=f"bA{4 + (it % 2)}")
                            S = a2.tile([128, CH], F32R,
                                        name=f"{p}S_{b}_{hq}_{qc}", tag="S",
                                        bufs=2)

                            def sc_mm(tt):
                                # 4-bank rotation so the trailing exp reads
                                # never gate the next phase's allocations
                                sc_ps = psA.tile([128, CH], F32,
                                                 name=f"{p}sc_{b}_{hq}_{qc}_{tt}",
                                                 tag=f"bA{(0, 1, 6, 7)[tt % 4]}")
                                nc.tensor.matmul(
                                    sc_ps[:],
                                    kT[:, b * T + tt * 128: b * T + (tt + 1) * 128],
                                    qT[:, hq * TOK + b * T + qc * CH:
                                       hq * TOK + b * T + (qc + 1) * CH],
                                    start=True, stop=True)
                                ex = ex_t[:, tt * CH:(tt + 1) * CH]
                                nc.scalar.activation(
                                    ex, sc_ps[:], mybir.ActivationFunctionType.Exp,
                                    bias=exp_bias[:], scale=ISCALE)
                                return ex

                            def cz_mm(tt, ex):
                                nc.tensor.matmul(
                                    ctx_ps[:],
                                    vN[:, (b * NT + tt) * 128:(b * NT + tt + 1) * 128],
                                    ex, start=(tt == 0), stop=(tt == NT - 1))

                            # fp16 pair-tree for the t-tile sums: 6 fp16 adds
                            # at DVE 2x rate + one f32r root add; keeps the
                            # DVE under the PE's per-iteration time
                            st_t = [a2.tile([128, CH], FP16,
                                            name=f"{p}st_{b}_{hq}_{qc}_{i}",
                                            tag="st", bufs=6)
                                    for i in range(2)]

                            def s_tree(i, x0, x1, exs):
                                # i: 0..3 pair index over (2i, 2i+1)
                                if i in (0, 2):
                                    nc.vector.tensor_add(st_t[i // 2][:], x0, x1)
                                elif i == 1:
                                    nc.vector.tensor_add(st_t[0][:],
                                                         st_t[0][:], x0)
                                    nc.vector.tensor_add(st_t[0][:],
                                                         st_t[0][:], x1)
                                else:
                                    nc.vector.tensor_add(st_t[1][:],
                                                         st_t[1][:], x0)
                                    nc.vector.tensor_add(st_t[1][:],
                                                         st_t[1][:], x1)
                                    nc.vector.tensor_add(S[:], st_t[0][:],
                                                         st_t[1][:])

                            # software pipeline: scores(tt) ahead of ctx(tt-1)
                            exs = [sc_mm(0)]
                            exs.append(sc_mm(1))
                            if pending is not None:
                                pending()
                                pending = None
                            cz_mm(0, exs[0])
                            for tt in range(2, NT):
                                exs.append(sc_mm(tt))
                                cz_mm(tt - 1, exs[tt - 1])
                                if tt % 2 == 1:
                                    s_tree(tt // 2 - 1, exs[tt - 3], exs[tt - 2],
                                           exs)
                            cz_mm(NT - 1, exs[NT - 1])
                            s_tree(3, exs[6], exs[7], exs)

                            def tail(b=b, hq=hq, qc=qc, it=it, ctx_ps=ctx_ps,
                                     S=S):
                                # Z = J.T @ S: partition-sum of S broadcast to
                                # all 128 partitions in one f32r pass
                                zb_ps = psA.tile([128, CH], F32,
                                                 name=f"{p}zb_{b}_{hq}_{qc}",
                                                 tag=f"bA{2 + (it % 2)}")
                                nc.tensor.matmul(zb_ps[:], ones_J[:], S[:],
                                                 start=True, stop=True)
                                recip = a2.tile([128, CH], F32,
                                                name=f"{p}rc_{b}_{hq}_{qc}",
                                                tag="recip")
                                nc.vector.reciprocal(recip[:], zb_ps[:])
                                ctxn = a2.tile([128, CH], BF16,
                                               name=f"{p}ctxn_{b}_{hq}_{qc}",
                                               tag="ctxn")
                                nc.vector.tensor_mul(ctxn[:], ctx_ps[:], recip[:])
                                nc.sync.dma_start(
                                    out=ag_in[b][qc][hq * 128:(hq + 1) * 128, :],
                                    in_=ctxn[:])

                            pending = tail
                            if hq == HQ - 1:
                                # half-batch context complete: flush the tail
                                # and launch this q-chunk's gather immediately
                                pending()
                                pending = None
                                if use_collective:
                                    nc.gpsimd.collective_compute(
                                        "AllGather",
                                        mybir.AluOpType.bypass,
                                        replica_groups=[list(range(NCORE))],
                                        ins=[ag_in[b][qc].opt()],
                                        outs=[ag_out[b][qc].opt()],
                                    )
                                else:
                                    # single-core timing stand-in: tiny copy to
                                    # create the dependency edge (the real
                                    # AllGather runs on TOPSP/SDMA silicon,
                                    # not the engine DMA queues)
                                    nc.sync.dma_start(
                                        out=ag_out[b][qc][0:16, 0:16],
                                        in_=ag_in[b][qc][0:16, 0:16])
                    if rp == 0 and b == 0:
                        nc.sync.dma_start(out=wo_sb[:], in_=wo[:])
                    if pipelined_oproj and b >= 1:
                        # o_proj pipelined one batch behind: spreads the 33MB
                        # gathered-ctx DMA across the run and gives each
                        # AllGather a full batch of slack
                        emit_oproj(p, b - 1, ag_out)
                if pipelined_oproj:
                    emit_oproj(p, B - 1, ag_out)
                else:
                    for b in range(B):
                        emit_oproj(p, b, ag_out)

    nc.compile()
    return nc


def _tile_w(w):
    """[H, O] row-major -> [128, HT*O] so [:, ht*O:(ht+1)*O] is rows ht*128..+128."""
    Hh, O = w.shape
    return np.ascontiguousarray(
        w.reshape(Hh // 128, 128, O).transpose(1, 0, 2).reshape(128, (Hh // 128) * O)
    ).astype(NPBF)


def _tile_act(xT):
    """[H, TOK] -> [128, NCH*HT*CH]: chunk-major tiling so each token-chunk's
    32 contraction tiles are contiguous (xT[ht*128+p, ch*CH+c] lands at
    [p, (ch*HT+ht)*CH + c])."""
    return np.ascontiguousarray(
        xT.reshape(HT, 128, NCH, CH).transpose(1, 2, 0, 3)
        .reshape(128, NCH * HT * CH)
    ).astype(NPBF)


def make_in_maps(inputs):
    hs = np.asarray(inputs["hidden_states"], np.float32).reshape(TOK, H)
    tr = np.asarray(inputs["traces"], np.float32).reshape(TOK, H)
    hsT = _tile_act(hs.T.astype(np.float32))
    trT = _tile_act(tr.T.astype(np.float32))

    def eff(Wname, Aname, Bname):
        W = np.asarray(inputs[Wname], np.float32)
        A = np.asarray(inputs[Aname], np.float32)
        Bm = np.asarray(inputs[Bname], np.float32)
        return W + np.float32(SCALING) * (A @ Bm)

    Wq = eff("Wq", "Aq", "Bq")
    Wk = eff("Wk", "Ak", "Bk")
    Wv = eff("Wv", "Av", "Bv")
    Wo = eff("Wo", "Ao", "Bo")

    in_maps = []
    for c in range(NCORE):
        in_maps.append({
            "hsT": hsT,
            "trT": trT,
            "wq": _tile_w(Wq[:, c * DQ:(c + 1) * DQ]),
            "wk": _tile_w(Wk[:, c * HD:(c + 1) * HD]),
            "wv": _tile_w(Wv[:, c * HD:(c + 1) * HD]),
            "wo": _tile_w(Wo[:, c * DQ:(c + 1) * DQ]),
            "hres": np.ascontiguousarray(hs[:, c * DQ:(c + 1) * DQ]),
        })
    return in_maps


_NC_CACHE = {}


def _get_runner():
    """Cached jitted 8-core runner (mirrors bass2jax.run_bass_via_pjrt but
    reuses the jit across kernel() calls)."""
    if "runner" in _NC_CACHE:
        return _NC_CACHE["runner"]
    import jax
    from jax.sharding import Mesh, PartitionSpec, NamedSharding
    from jax.experimental.shard_map import shard_map
    import concourse.mybir as mb
    from concourse import bass2jax

    nc = _NC_CACHE.get("nc")
    if nc is None:
        nc = _NC_CACHE["nc"] = build_nc(use_collective=True)
    bass2jax.install_neuronx_cc_hook()
    partition_name = nc.partition_id_tensor.name if nc.partition_id_tensor else None
    in_names, out_names, out_avals, zero_outs = [], [], [], []
    for alloc in nc.m.functions[0].allocations:
        if not isinstance(alloc, mb.MemoryLocationSet):
            continue
        name = alloc.memorylocations[0].name
        if alloc.kind == "ExternalInput":
            if name != partition_name:
                in_names.append(name)
        elif alloc.kind == "ExternalOutput":
            out_names.append(name)
            shape = tuple(alloc.tensor_shape)
            dtype = mb.dt.np(alloc.dtype)
            out_avals.append(jax.core.ShapedArray(shape, dtype))
            zero_outs.append(np.zeros(shape, dtype))
    all_names = in_names + out_names
    if partition_name is not None:
        all_names = all_names + [partition_name]

    def _body(*args):
        operands = list(args)
        if partition_name is not None:
            operands.append(bass2jax.partition_id_tensor())
        outs = bass2jax._bass_exec_p.bind(
            *operands,
            out_avals=tuple(out_avals),
            in_names=tuple(all_names),
            out_names=tuple(out_names),
            lowering_input_output_aliases=(),
            sim_require_finite=True,
            sim_require_nnan=True,
            nc=nc,
        )
        return tuple(outs)

    devices = jax.devices()[:NCORE]
    mesh = Mesh(np.asarray(devices), ("core",))
    spec = PartitionSpec("core")
    fn = jax.jit(shard_map(_body, mesh=mesh,
                           in_specs=(spec,) * (len(in_names) + len(out_names)),
                           out_specs=(spec,) * len(out_names), check_rep=False))
    sharding = NamedSharding(mesh, spec)
    zeros_dev = [
        jax.device_put(np.zeros((NCORE * z.shape[0], *z.shape[1:]), z.dtype), sharding)
        for z in zero_outs
    ]
    runner = {"fn": fn, "in_names": in_names, "out_names": out_names,
              "zeros": zeros_dev, "sharding": sharding, "jax": jax}
    _NC_CACHE["runner"] = runner
    return runner


def kernel(**inputs) -> np.ndarray:
    r = _get_runner()
    in_maps = make_in_maps(inputs)
    jax = r["jax"]
    args = [
        jax.device_put(
            np.concatenate([np.asarray(m[name]) for m in in_maps], axis=0),
            r["sharding"])
        for name in r["in_names"]
    ] + r["zeros"]
    outs = r["fn"](*args)
    oi = r["out_names"].index("out")
    full = np.asarray(outs[oi]).reshape(NCORE, TOK, DQ)
    out_full = np.empty((TOK, H), np.float32)
    for c in range(NCORE):
        out_full[:, c * DQ:(c + 1) * DQ] = full[c]
    return out_full.reshape(B, T, H)
